# revision 20
# baseline (speedup 1.0000x reference)
# Trainium2 Bass kernel for nn_CrossAttention (8-core SPMD).
#
# Reference computation (fp32):
#   q = x @ Wq; k = ctx @ Wk; v = ctx @ Wv        (per-head d=64, 8 heads)
#   out = softmax(q k^T / sqrt(d)) v              (full attention)
#   y = out @ Wo + bo
#
# Sharding: 8 cores = 4 batches x 2 query-row halves. Each core gets one
# batch's context and half of that batch's 4096 query rows (2048 rows), all
# weights, and produces its full [2048, 1024] output slice independently —
# no collectives, host only concatenates.
#
# On-chip layout:
#   - inputs are cast to bf16 on the host (halves DMA traffic; all matmuls
#     run bf16 with fp32 PSUM accumulation).
#   - PE contracts over the partition dim, so x and context are fed
#     pre-transposed (xT: [1024, 2048] bf16, ctxT: [768, 1024] bf16).
#   - q,k are produced transposed (inner dim on partitions) which is exactly
#     the layout the scores matmul needs; v is produced natural [Skv, inner]
#     with a column of ones appended per head so the attention-PV matmul
#     also yields the softmax denominator for free.
#   - scores are computed transposed [Skv, Sq-chunk]; exp runs on ScalarE
#     straight out of PSUM; no row-max subtraction (scores are O(10)).
#   - the whole main loop is one static software pipeline: each head-pair
#     "body" issues its scores matmuls interleaved (in PE program order)
#     with the previous pair's PV matmuls, the next query-chunk's q
#     projection, and the previous chunk's output projection, so the PE
#     never idles waiting for the ScalarE exp stream (idle gaps re-throttle
#     the PE clock gate to 1.2 GHz, which is what limited the baseline).
#   - softmax denominators are handled per head-pair: two DVE row copies,
#     reciprocal_approx_fast, and a tiny K=2 selection matmul broadcast —
#     all off the critical path (the baseline's batched DMA-gather +
#     3.3us exact reciprocal stalled the PE every chunk).
import os

import numpy as np

import concourse.bass as bass
import concourse.tile as tile
from concourse import bacc, mybir
from concourse.bass_utils import run_bass_kernel_spmd

F32 = mybir.dt.float32
F32R = mybir.dt.float32r
BF16 = mybir.dt.bfloat16
EXP = mybir.ActivationFunctionType.Exp
P = 128

B = 4
SQ_FULL = 4096
SQ = 2048          # per-core query rows
SKV = 1024
DQ = 1024
DKV = 768
INNER = 512
H = 8
DH = 64
SQC = 512          # query-chunk (matmul free dim)
NSQ = SQ // SQC    # 4
KCQ = DQ // P      # 8
KCK = DKV // P     # 6
NIC = INNER // P   # 4 inner-dim partition chunks
NJ = SKV // P      # 8 key chunks
NHP = H // 2       # 4 head pairs
SCALE = 1.0 / 8.0  # dh ** -0.5


def _mm(nc, out, lhsT, rhs, start, stop):
    nc.tensor.matmul(out, lhsT, rhs, start=start, stop=stop)


def build_nc():
    nc = bacc.Bacc(trn_type="TRN2", target_bir_lowering=False, debug=False)
    xT = nc.dram_tensor("xT", [DQ, SQ], BF16, kind="ExternalInput").ap()
    ctxT = nc.dram_tensor("ctxT", [DKV, SKV], BF16, kind="ExternalInput").ap()
    Wq = nc.dram_tensor("Wq", [DQ, INNER], BF16, kind="ExternalInput").ap()
    Wk = nc.dram_tensor("Wk", [DKV, INNER], BF16, kind="ExternalInput").ap()
    Wv = nc.dram_tensor("Wv", [DKV, INNER], BF16, kind="ExternalInput").ap()
    Wo = nc.dram_tensor("Wo", [INNER, DQ], BF16, kind="ExternalInput").ap()
    bo = nc.dram_tensor("bo", [DQ], F32, kind="ExternalInput").ap()
    selm = nc.dram_tensor("selm", [2, P], BF16, kind="ExternalInput").ap()
    y = nc.dram_tensor("y", [SQ, DQ], F32, kind="ExternalOutput").ap()

    with tile.TileContext(nc) as tc:
        with (
            tc.tile_pool(name="res", bufs=1) as res,
            tc.tile_pool(name="setup", bufs=1) as setup,
            tc.tile_pool(name="xq", bufs=2) as xq,
            tc.tile_pool(name="qt", bufs=2) as qt,
            tc.tile_pool(name="ex", bufs=2) as ex,
            tc.tile_pool(name="at", bufs=2) as at,
            tc.tile_pool(name="yp", bufs=2) as yp,
            tc.tile_pool(name="rc", bufs=2) as rc,
            tc.tile_pool(name="mmps", bufs=2, space="PSUM") as mmps,
            tc.tile_pool(name="scps", bufs=1, space="PSUM") as scps,
        ):
            # --- input DMAs, in setup-critical-path order ---
            # ctx is split in two skv-halves so the first kT matmuls can
            # start as soon as the first half + Wk land
            ctx_sb = setup.tile([P, KCK, SKV], BF16)
            nc.sync.dma_start(
                ctx_sb[:, :, 0:512],
                ctxT[:, 0:512].rearrange("(c p) m -> p c m", p=P))
            Wk_sb = setup.tile([P, KCK, INNER], BF16)
            nc.sync.dma_start(Wk_sb[:], Wk.rearrange("(c p) m -> p c m", p=P))
            nc.sync.dma_start(
                ctx_sb[:, :, 512:1024],
                ctxT[:, 512:1024].rearrange("(c p) m -> p c m", p=P))
            Wq_sb = res.tile([P, KCQ, INNER], BF16)
            nc.sync.dma_start(Wq_sb[:], Wq.rearrange("(c p) m -> p c m", p=P))

            xT_tiles = {}

            def load_xT(s):
                t = xq.tile([P, KCQ, SQC], BF16, name=f"xT{s}", tag="xT")
                nc.sync.dma_start(
                    t[:],
                    xT[:, s * SQC:(s + 1) * SQC]
                    .rearrange("(c p) m -> p c m", p=P),
                )
                xT_tiles[s] = t

            load_xT(0)
            Wv_sb = setup.tile([P, KCK, INNER], BF16)
            nc.sync.dma_start(Wv_sb[:], Wv.rearrange("(c p) m -> p c m", p=P))
            Wo_sb = res.tile([P, NIC, DQ], BF16)
            nc.sync.dma_start(Wo_sb[:], Wo.rearrange("(c p) m -> p c m", p=P))
            bo_sb = res.tile([P, DQ], F32)
            nc.sync.dma_start(bo_sb[:], bo.unsqueeze(0).broadcast_to([P, DQ]))
            selm_sb = res.tile([1, 2, P], BF16)
            nc.sync.dma_start(selm_sb[:], selm.unsqueeze(0))
            load_xT(1)

            kT_sb = res.tile([P, NIC, SKV], BF16)
            v_sb = res.tile([P, NJ, H, DH + 1], BF16)
            ones_sb = res.tile([P, NJ * H], BF16)
            nc.vector.memset(ones_sb[:], 1.0)
            nc.vector.tensor_copy(
                v_sb[:, :, :, DH:DH + 1],
                ones_sb.rearrange("p (a b u) -> p a b u", a=NJ, u=1),
            )

            # --- setup MMs: kT from context ---
            # kT[ic*128:+128, nk*512:+512] = Wk[:, ic-chunk].T @ ctxT[:, nk-chunk]
            for nk in range(SKV // 512):
                for ic in range(NIC):
                    ps = mmps.tile([P, 512], F32, tag="qy")
                    for kc in range(KCK):
                        _mm(nc, ps[:], Wk_sb[:, kc, ic * P:(ic + 1) * P],
                            ctx_sb[:, kc, nk * 512:(nk + 1) * 512],
                            kc == 0, kc == KCK - 1)
                    nc.vector.tensor_copy(kT_sb[:, ic, nk * 512:(nk + 1) * 512], ps[:])

            qT_tiles = {}

            def qT_group(s, ic):
                # 8 matmuls projecting one inner-chunk of q for chunk s
                if ic == 0:
                    qT_tiles[s] = qt.tile([P, NIC, SQC], BF16, name=f"qT{s}", tag="qT")
                xT_sb = xT_tiles[s]
                ps = mmps.tile([P, SQC], F32, tag="qy", name=f"qp{s}_{ic}")
                for kc in range(KCQ):
                    _mm(nc, ps[:], Wq_sb[:, kc, ic * P:(ic + 1) * P],
                        xT_sb[:, kc, :], kc == 0, kc == KCQ - 1)
                nc.vector.tensor_copy(qT_tiles[s][:, ic, :], ps[:])

            # q projection for chunk 0 (x0 DMA lands while kT matmuls run)
            for ic in range(NIC):
                qT_group(0, ic)

            # v natural: v[j*128:+128, :] = ctxT[:, j-chunk].T @ Wv
            # (deferred: issued as PE filler inside the first attention body)
            def v_group(j):
                ps = mmps.tile([P, INNER], F32, tag="pv", name=f"vp{j}")
                for kc in range(KCK):
                    _mm(nc, ps[:], ctx_sb[:, kc, j * P:(j + 1) * P],
                        Wv_sb[:, kc, :], kc == 0, kc == KCK - 1)
                nc.vector.tensor_copy(
                    v_sb[:, j, :, 0:DH],
                    ps.rearrange("p (h d) -> p h d", h=H),
                )

            attn_tiles = {}
            pend = None  # (s, hp, exp0, exp1) awaiting PV + normalize

            def pv_head(s, hp, po, expb):
                # PV for one head: [dh+1, SQC]; row dh = softmax denominator
                par = po // DH
                h = 2 * hp + par
                pv = mmps.tile([P, SQC], F32, tag="pv", name=f"pv{s}_{h}")
                for j in range(NJ):
                    _mm(nc, pv[0:DH + 1, :], v_sb[:, j, h, :],
                        expb[:, j, par, :], j == 0, j == NJ - 1)
                nc.vector.tensor_copy(
                    attn_tiles[s][po:po + DH, hp, :], pv[0:DH, :])
                return pv

            def yo_group(s, r):
                # output projection + bias for one 128-row slice of chunk s
                y_sb = yp.tile([P, DQ], F32, tag="y", name=f"y{s}_{r}")
                for nh in range(DQ // 512):
                    ps = mmps.tile([P, 512], F32, tag="qy", name=f"yp{s}_{r}_{nh}")
                    for kc in range(NIC):
                        _mm(nc, ps[:],
                            attn_tiles[s][:, kc, r * P:(r + 1) * P],
                            Wo_sb[:, kc, nh * 512:(nh + 1) * 512],
                            kc == 0, kc == NIC - 1)
                    nc.vector.tensor_add(
                        y_sb[:, nh * 512:(nh + 1) * 512], ps[:],
                        bo_sb[:, nh * 512:(nh + 1) * 512])
                nc.sync.dma_start(
                    y[s * SQC + r * P: s * SQC + (r + 1) * P, :], y_sb[:])

            def pend_pv(jg):
                # previous head-pair's PV + denominator path, split by jg
                if pend is None:
                    return
                ps_, hp_, expb_ = pend
                if jg == 0:
                    pv = pv_head(ps_, hp_, 0, expb_)
                    den2 = rc.tile([1, 2, SQC], BF16, tag="den", name=f"dn{ps_}_{hp_}")
                    nc.vector.tensor_copy(den2[0:1, 0, :], pv[DH:DH + 1, :])
                    pend_den[0] = den2
                elif jg == 1:
                    pv = pv_head(ps_, hp_, DH, expb_)
                    den2 = pend_den[0]
                    nc.vector.tensor_copy(den2[0:1, 1, :], pv[DH:DH + 1, :])
                elif jg == 3:
                    # broadcast den over each head's 64-partition block (two
                    # accumulating K=1 matmuls; selm row po has ones on
                    # partitions [po*64, po*64+64)), then reciprocal across
                    # all 128 lanes and normalize that inner-chunk of attn
                    den2 = pend_den[0]
                    rps = mmps.tile([P, SQC], F32, tag="pv", name=f"rb{ps_}_{hp_}")
                    _mm(nc, rps[:], selm_sb[0:1, 0, :], den2[0:1, 0, :],
                        True, False)
                    _mm(nc, rps[:], selm_sb[0:1, 1, :], den2[0:1, 1, :],
                        False, True)
                    rrec = rc.tile([P, SQC], F32, tag="rrec", name=f"rr{ps_}_{hp_}")
                    nc.vector.reciprocal_approx_fast(rrec[:], rps[:])
                    nc.vector.tensor_mul(
                        attn_tiles[ps_][:, hp_, :],
                        attn_tiles[ps_][:, hp_, :], rrec[:])

            pend_den = [None, None]

            # --- main pipeline over (query chunk, head pair) bodies ---
            for s in range(NSQ):
                if s + 2 < NSQ:
                    load_xT(s + 2)
                attn_tiles[s] = at.tile([P, NIC, SQC], BF16, name=f"at{s}", tag="at")
                qT_sb = qT_tiles[s]
                for hp in range(NHP):
                    # merged exp tile: [skv-part, j, head-parity, sq]
                    expb = ex.tile([P, NJ, 2, SQC], BF16, tag="expb",
                                   name=f"eb{s}_{hp}")
                    for jg in range(NJ // 2):
                        # scores^T [Skv, SQC] for the head pair; the two K=64
                        # matmuls per j land on disjoint PE row groups. All
                        # four land in ONE 4-bank PSUM tile drained by ONE
                        # exp call, so all four become ready at the same
                        # instant and the scheduler keeps them contiguous —
                        # which is what lets the A/B row groups overlap.
                        sps = scps.tile([P, 4, 512], F32, tag="sc",
                                        name=f"sc{s}_{hp}_{jg}")
                        for jj in range(2):
                            j = jg * 2 + jj
                            _mm(nc, sps[:, 2 * jj, :],
                                kT_sb[0:DH, hp, j * P:(j + 1) * P],
                                qT_sb[0:DH, hp, :], True, True)
                            _mm(nc, sps[:, 2 * jj + 1, :],
                                kT_sb[DH:P, hp, j * P:(j + 1) * P],
                                qT_sb[DH:P, hp, :], True, True)
                        nc.scalar.activation(
                            expb[:, jg * 2:jg * 2 + 2, :, :], sps[:],
                            EXP, scale=SCALE)
                        # PE filler between score groups: keeps the PE queue
                        # deep enough that it never idles on the exp stream
                        if jg == 0:
                            pend_pv(0)
                            if s == 0 and hp == 0:
                                for j in range(4):
                                    v_group(j)
                        elif jg == 1:
                            pend_pv(1)
                            if s == 0 and hp == 0:
                                for j in range(4, NJ):
                                    v_group(j)
                        elif jg == 2:
                            if s + 1 < NSQ:
                                qT_group(s + 1, hp)
                        elif jg == 3:
                            pend_pv(3)
                            if s > 0:
                                yo_group(s - 1, hp)
                    pend = (s, hp, expb)

            # --- epilogue: last head pair + last chunk's output projection ---
            pend_pv(0)
            pend_pv(1)
            pend_pv(3)
            for r in range(NHP):
                yo_group(NSQ - 1, r)
    nc.compile()
    return nc


_NC_CACHE = None


def kernel(x, context, Wq, Wk, Wv, Wo, bo):
    global _NC_CACHE
    import ml_dtypes
    bf16 = ml_dtypes.bfloat16

    x = np.asarray(x, dtype=np.float32)
    context = np.asarray(context, dtype=np.float32)
    Wq_b = np.ascontiguousarray(np.asarray(Wq, dtype=np.float32).astype(bf16))
    Wk_b = np.ascontiguousarray(np.asarray(Wk, dtype=np.float32).astype(bf16))
    Wv_b = np.ascontiguousarray(np.asarray(Wv, dtype=np.float32).astype(bf16))
    Wo_b = np.ascontiguousarray(np.asarray(Wo, dtype=np.float32).astype(bf16))
    bo = np.ascontiguousarray(np.asarray(bo, dtype=np.float32))

    if _NC_CACHE is None:
        _NC_CACHE = build_nc()
    nc = _NC_CACHE

    selm = np.zeros((2, P), dtype=np.float32)
    for po in range(2):
        selm[po, po * DH:(po + 1) * DH] = 1.0
    selm = np.ascontiguousarray(selm.astype(bf16))

    in_maps = []
    for c in range(8):
        b, half = c // 2, c % 2
        xs = x[b, half * SQ:(half + 1) * SQ, :]            # [2048, 1024]
        in_maps.append({
            "xT": np.ascontiguousarray(xs.T.astype(bf16)),       # [1024, 2048]
            "ctxT": np.ascontiguousarray(context[b].T.astype(bf16)),  # [768, 1024]
            "Wq": Wq_b, "Wk": Wk_b, "Wv": Wv_b, "Wo": Wo_b, "bo": bo,
            "selm": selm,
        })

    trace = bool(int(os.environ.get("KERNEL_TRACE", "0")))
    res = run_bass_kernel_spmd(nc, in_maps, core_ids=list(range(8)), trace=trace)
    kernel.last_results = res

    out = np.empty((B, SQ_FULL, DQ), dtype=np.float32)
    for c in range(8):
        b, half = c // 2, c % 2
        out[b, half * SQ:(half + 1) * SQ, :] = res.results[c]["y"]
    return out
